# revision 8
# baseline (speedup 1.0000x reference)
"""Fused multi-head tanh-attention kernel for Trainium2 (8 NeuronCores).

Problem: y[s,b,:] = concat_h( softmax_t(tanh(q_h k_h^T / 8) - 10000*(1-mask)) @ v_h )
with q/k/v = per-head projections of x.  Shapes: x [1024,16,512], mask [16,1024],
w* [8,64,512] -> y [1024,16,512].

Strategy: batch-parallel over 8 cores (2 batches per core).  Per core, a fully
fused flash-style pipeline keeps the [S,S] score matrices in PSUM/SBUF only:
  - x and the weights are transposed on-chip via PE-transpose (contraction dim
    on partitions), all matmuls run in float32r (full PE rate, ~1e-4 rel err),
  - scores are built in scoresT [t,s] layout so the mask is a per-partition
    bias on the Exp activation and the softmax denominator falls out of the
    PV matmul via an appended ones-column on v,
  - tanh+exp run on the scalar engine (one table set holds both), PV
    accumulates unnormalized out^T, which is PE-transposed back and divided
    by the denominator on the vector engine before the store.
"""

import sys

sys.path.insert(0, "/opt/trn_rl_repo")

from contextlib import ExitStack

import numpy as np

S, B, D, H, DH = 1024, 16, 512, 8, 64
NCORES = 8
BPC = B // NCORES  # batches per core
SC = S // 128  # 8 s-chunks (and t-chunks)
DC = D // 128  # 4 d-chunks

_compiled_nc = None


def _make_pools(tc, ctx):
    pools = {}
    pools["singles"] = ctx.enter_context(tc.tile_pool(name="singles", bufs=1))
    pools["nat"] = ctx.enter_context(tc.tile_pool(name="nat", bufs=3))
    pools["qk"] = ctx.enter_context(tc.tile_pool(name="qk", bufs=4))
    pools["vh"] = ctx.enter_context(tc.tile_pool(name="vh", bufs=10))
    pools["tanh"] = ctx.enter_context(tc.tile_pool(name="tanh", bufs=3))
    pools["exp"] = ctx.enter_context(tc.tile_pool(name="exp", bufs=3))
    pools["outT"] = ctx.enter_context(tc.tile_pool(name="outT", bufs=2))
    pools["outsb"] = ctx.enter_context(tc.tile_pool(name="outsb", bufs=2))
    pools["small"] = ctx.enter_context(tc.tile_pool(name="small", bufs=4))
    pools["ps_big"] = ctx.enter_context(tc.tile_pool(name="ps_big", bufs=3, space="PSUM"))
    pools["ps_o"] = ctx.enter_context(tc.tile_pool(name="ps_o", bufs=1, space="PSUM"))
    return pools


def _emit(nc, tc, pools, tile, mybir, aps, u=0):
    f32 = mybir.dt.float32
    f32r = mybir.dt.float32r
    AF = mybir.ActivationFunctionType
    Alu = mybir.AluOpType
    x_d, mask_d, wq_d, wk_d, wv_d, id_d, y_d = aps

    singles = pools["singles"]
    nat = pools["nat"]
    qk_pool = pools["qk"]
    vh_pool = pools["vh"]
    tanh_pool = pools["tanh"]
    exp_pool = pools["exp"]
    outT_pool = pools["outT"]
    outsb_pool = pools["outsb"]
    small = pools["small"]
    ps_big = pools["ps_big"]
    ps_o = pools["ps_o"]

    ident = singles.tile([128, 128], f32r, tag="ident", name=f"ident_u{u}")
    nc.sync.dma_start(ident, id_d)
    fill64 = singles.tile([128, SC, 64], f32, tag="fill64", name=f"fill64_u{u}")
    nc.vector.memset(fill64, 1.0)

    # ---- weights: transpose [e,d] -> [d,e] layouts on PE ----------------
    # q/k: per head-pair hp, per d-chunk: [128(d), 128(2 heads x 64 e)]
    wTq = {}
    wTk = {}
    for w_d, wT, nm in ((wq_d, wTq, "q"), (wk_d, wTk, "k")):
        for hp in range(H // 2):
            w_nat = nat.tile([128, D], f32r, tag="nat", name=f"w_nat_u{u}")
            nc.sync.dma_start(
                w_nat, w_d[2 * hp : 2 * hp + 2].rearrange("h e d -> (h e) d")
            )
            wt = singles.tile([128, DC, 128], f32r, tag=f"wT{nm}{hp}", name=f"wT{nm}{hp}_u{u}")
            wT[hp] = wt
            for dc in range(DC):
                pst = ps_big.tile([128, 128], f32r, tag="ps_big", name=f"pstr_u{u}")
                nc.tensor.transpose(pst, w_nat[:, dc * 128 : dc * 128 + 128], ident)
                nc.vector.tensor_copy(wt[:, dc, :], pst)
    # v: per head-quad: [128(d), 4, 256(4 heads x 64 e)]
    wTv = {}
    for q4 in range(H // 4):
        wt = singles.tile([128, DC, 256], f32r, tag=f"wTv{q4}", name=f"wTv{q4}_u{u}")
        wTv[q4] = wt
        for half in range(2):
            w_nat = nat.tile([128, D], f32r, tag="nat", name=f"w_nat_u{u}")
            h0 = 4 * q4 + 2 * half
            nc.sync.dma_start(w_nat, wv_d[h0 : h0 + 2].rearrange("h e d -> (h e) d"))
            for dc in range(DC):
                pst = ps_big.tile([128, 128], f32r, tag="ps_big", name=f"pstr_u{u}")
                nc.tensor.transpose(pst, w_nat[:, dc * 128 : dc * 128 + 128], ident)
                nc.vector.tensor_copy(wt[:, dc, half * 128 : half * 128 + 128], pst)

    # ---- mask -> per-partition exp bias: bias[t] = (mask[t]-1)*10000 ----
    mask_bias = {}
    for b in range(BPC):
        msk = small.tile([128, SC], f32, tag="msk", name=f"msk_u{u}")
        nc.sync.dma_start(msk, mask_d[b].rearrange("(c p) -> p c", p=128))
        bias = singles.tile([128, SC], f32, tag=f"bias{b}", name=f"bias{b}_u{u}")
        mask_bias[b] = bias
        nc.vector.tensor_scalar(bias, msk, -1.0, 10000.0, Alu.add, Alu.mult)

    # ---- x: load + PE-transpose to x^T [d, s] per batch -----------------
    xbT = {}
    for b in range(BPC):
        for dc in range(DC):
            xbT[b, dc] = singles.tile([128, S], f32r, tag=f"xbT{b}{dc}", name=f"xbT{b}{dc}_u{u}")
        for sc in range(SC):
            x_nat = nat.tile([128, D], f32r, tag="nat", name=f"x_nat_u{u}")
            nc.sync.dma_start(x_nat, x_d[sc * 128 : sc * 128 + 128, b, :])
            for dc in range(DC):
                pst = ps_big.tile([128, 128], f32r, tag="ps_big", name=f"pstr_u{u}")
                nc.tensor.transpose(pst, x_nat[:, dc * 128 : dc * 128 + 128], ident)
                nc.vector.tensor_copy(xbT[b, dc][:, sc * 128 : sc * 128 + 128], pst)

    # ---- main loop: per batch, per head-pair: project then attend -------
    yr = y_d.rearrange("(c p) b e -> p c b e", p=128)
    for b in range(BPC):
        vh = {}
        for hp in range(H // 2):
            # projections q^T,k^T [128(2 heads x e), S] for this head pair
            qkT = {}
            for wT, nm in ((wTq, "q"), (wTk, "k")):
                psp = ps_big.tile([128, S], f32, tag="ps_big", name=f"psp_u{u}")
                for dc in range(DC):
                    for sh in range(2):
                        nc.tensor.matmul(
                            psp[:, sh * 512 : sh * 512 + 512],
                            wT[hp][:, dc, :],
                            xbT[b, dc][:, sh * 512 : sh * 512 + 512],
                            start=(dc == 0),
                            stop=(dc == DC - 1),
                        )
                t = qk_pool.tile([128, S], f32r, tag="qkT", name=f"qkT{nm}_u{u}")
                qkT[nm] = t
                nc.vector.tensor_copy(t, psp)
            # v in natural [t, e] layout, one head-quad at a time
            if hp % 2 == 0:
                q4 = hp // 2
                for h_in, h in enumerate(range(4 * q4, 4 * q4 + 4)):
                    vh[h] = vh_pool.tile([128, SC, 128], f32r, tag="vh", name=f"vh{b}_{h}_u{u}")
                    nc.vector.tensor_copy(vh[h][:, :, 64:128], fill64)
                for tck in range(SC):
                    psv = ps_big.tile([128, 256], f32, tag="ps_big", name=f"psv_u{u}")
                    for dc in range(DC):
                        nc.tensor.matmul(
                            psv,
                            xbT[b, dc][:, tck * 128 : tck * 128 + 128],
                            wTv[q4][:, dc, :],
                            start=(dc == 0),
                            stop=(dc == DC - 1),
                        )
                    for h_in, h in enumerate(range(4 * q4, 4 * q4 + 4)):
                        nc.vector.tensor_copy(
                            vh[h][:, tck, 0:64], psv[:, h_in * 64 : h_in * 64 + 64]
                        )
            # attention for the two heads of this pair
            for h2 in range(2):
                h = 2 * hp + h2
                r0 = h2 * 64
                pso = ps_o.tile([128, S], f32, tag="ps_o", name=f"pso_u{u}")
                for tck in range(SC):
                    pss = ps_big.tile([128, S], f32, tag="ps_big", name=f"pss_u{u}")
                    for sh in range(2):
                        nc.tensor.matmul(
                            pss[:, sh * 512 : sh * 512 + 512],
                            qkT["k"][r0 : r0 + 64, tck * 128 : tck * 128 + 128],
                            qkT["q"][r0 : r0 + 64, sh * 512 : sh * 512 + 512],
                            start=True,
                            stop=True,
                        )
                    tnh = tanh_pool.tile([128, S], f32, tag="tanh", name=f"tnh_u{u}")
                    nc.scalar.activation(tnh, pss, AF.Tanh, scale=0.125)
                    ex = exp_pool.tile([128, S], f32r, tag="exp", name=f"ex_u{u}")
                    nc.scalar.activation(
                        ex, tnh, AF.Exp, bias=mask_bias[b][:, tck : tck + 1]
                    )
                    for sh in range(2):
                        nc.tensor.matmul(
                            pso[:, sh * 512 : sh * 512 + 512],
                            vh[h][:, tck, :],
                            ex[:, sh * 512 : sh * 512 + 512],
                            start=(tck == 0),
                            stop=(tck == SC - 1),
                        )
                outT = outT_pool.tile([128, S], f32r, tag="outT", name=f"outT_u{u}")
                nc.vector.tensor_copy(outT, pso)
                # transpose back to [s, e] + normalize by the ones-column sum
                pst = ps_big.tile([128, SC, 128], f32r, tag="ps_big", name=f"psto_u{u}")
                for sc in range(SC):
                    nc.tensor.transpose(
                        pst[:, sc, 0:128],
                        outT[:, sc * 128 : sc * 128 + 128],
                        ident,
                    )
                rec = small.tile([128, SC], f32, tag="rec", name=f"rec_u{u}")
                nc.vector.reciprocal(rec, pst[:, :, 64])
                osb = outsb_pool.tile([128, SC, 64], f32, tag="osb", name=f"osb_u{u}")
                for sc in range(SC):
                    nc.vector.tensor_scalar(
                        osb[:, sc, :],
                        pst[:, sc, 0:64],
                        rec[:, sc : sc + 1],
                        None,
                        Alu.mult,
                    )
                nc.sync.dma_start(yr[:, :, b, h * 64 : h * 64 + 64], osb)


def _build(unroll=1):
    import concourse.bass as bass  # noqa: F401
    import concourse.tile as tile
    from concourse import bacc, mybir

    f32 = mybir.dt.float32
    f32r = mybir.dt.float32r
    nc = bacc.Bacc("TRN2", target_bir_lowering=False, debug=False)
    x_d = nc.dram_tensor("x", [S, BPC, D], f32r, kind="ExternalInput").ap()
    mask_d = nc.dram_tensor("mask", [BPC, S], f32, kind="ExternalInput").ap()
    wq_d = nc.dram_tensor("wq", [H, DH, D], f32r, kind="ExternalInput").ap()
    wk_d = nc.dram_tensor("wk", [H, DH, D], f32r, kind="ExternalInput").ap()
    wv_d = nc.dram_tensor("wv", [H, DH, D], f32r, kind="ExternalInput").ap()
    id_d = nc.dram_tensor("ident", [128, 128], f32r, kind="ExternalInput").ap()
    y_d = nc.dram_tensor("y", [S, BPC, D], f32, kind="ExternalOutput").ap()
    with tile.TileContext(nc) as tc, ExitStack() as ctx:
        pools = _make_pools(tc, ctx)
        for u in range(unroll):
            _emit(nc, tc, pools, tile, mybir, (x_d, mask_d, wq_d, wk_d, wv_d, id_d, y_d), u)
    nc.compile()
    return nc


def get_compiled():
    global _compiled_nc
    if _compiled_nc is None:
        _compiled_nc = _build()
    return _compiled_nc


def make_in_maps(x, mask, wq, wk, wv):
    x = np.asarray(x, np.float32)
    mask = np.asarray(mask, np.float32)
    wq = np.ascontiguousarray(np.asarray(wq, np.float32))
    wk = np.ascontiguousarray(np.asarray(wk, np.float32))
    wv = np.ascontiguousarray(np.asarray(wv, np.float32))
    ident = np.eye(128, dtype=np.float32)
    maps = []
    for c in range(NCORES):
        maps.append(
            {
                "x": np.ascontiguousarray(x[:, c * BPC : (c + 1) * BPC, :]),
                "mask": np.ascontiguousarray(mask[c * BPC : (c + 1) * BPC, :]),
                "wq": wq,
                "wk": wk,
                "wv": wv,
                "ident": ident,
            }
        )
    return maps


def kernel(x, mask, wq, wk, wv):
    from concourse.bass_utils import run_bass_kernel_spmd

    nc = get_compiled()
    in_maps = make_in_maps(x, mask, wq, wk, wv)
    res = run_bass_kernel_spmd(nc, in_maps, list(range(NCORES))).results
    y = np.concatenate([r["y"] for r in res], axis=1)
    return np.ascontiguousarray(y.astype(np.float32, copy=False))
